# revision 55
# baseline (speedup 1.0000x reference)
"""Trainium2 Bass kernel for the proxy-anchor style supervised-contrastive loss.

Problem (hardcoded): B=2048, D=128, C=8 classes, P=32 proxies, N=2B+P=4128,
TEMPERATURE=0.1.  loss = -mean_i[ sum_{j~i,j!=i}(logits_ij)/cnt_i' adjustments
                                   - log(sum_{j!=i} exp(logits_ij - shift_i) * w_ij) ]
with logits = feats @ feats.T / T and w_ij = 1/(cls_count[t_j] - mask_ij).

Sharding: the 4128 rows of the (never materialized) 4128x4128 logits matrix
are split across 8 NeuronCores, 516 rows each, processed as 5 chunks of <=128
rows.  Each core holds all features (feats.T resident in SBUF, sorted by
class), computes its row-block of logits class-pair by class-pair in PSUM
(fp32r fast path), applies exp with a per-row bias (diagonal shift) on the
scalar engine — one call per class-pair PSUM tile, with the free running-sum
(accum_out) giving pair sums — disentangles per-class sums with one vector
reduce per pair, extracts the diagonal value through 8 stride-516 windows
masked by a per-device selector, and emits two f32 values per row: the
picked-class logit sum (times 1/(cnt-1)) and the softmax denominator.  The
host applies the final log / mean, reproducing the reference's fp32 NaN/inf
semantics exactly: for randn-scale inputs every off-diagonal exp underflows
to exactly 0 and all on-chip sums cancel bitwise, so denominators are exact
zeros, log gives -inf and the masked mean gives NaN just like the reference.

Self-contained: only numpy + the concourse (Bass) runtime are imported.
"""

import numpy as np

PROXY_NUM_LIST = [8, 6, 5, 4, 3, 3, 2, 1]
TEMPERATURE = 0.1
B, D, C = 2048, 128, 8
NPROXY = 32
N = 2 * B + NPROXY            # 4128
NCORES = 8
RP = N // NCORES              # 516 rows per core
NCH = 5                       # row chunks per core (5 x 128 = 640 slots)
RPAD = NCH * 128              # 640

_prog_cache: dict = {}


def _build_program(offs, fp32r=True, stage=4):
    """Build the SPMD Bass program.  `offs` = class segment boundaries
    (9 ints, offs[0]=0, offs[8]=N) — identical on every core."""
    from contextlib import ExitStack

    import concourse.bacc as bacc
    import concourse.bass as bass
    import concourse.mybir as mybir
    import concourse.tile as tile

    f32 = mybir.dt.float32
    i32 = mybir.dt.int32
    ALU = mybir.AluOpType
    ACT = mybir.ActivationFunctionType
    AX = mybir.AxisListType

    nc = bacc.Bacc(
        "TRN2",
        target_bir_lowering=False,
        debug=False,
        enable_asserts=False,
        num_devices=NCORES,
    )

    mmdt = mybir.dt.float32r if fp32r else f32
    # fmat = [ lhsT10 (RPAD) | GT (C) | featsT (N) | pad (2) ]
    FM = RPAD + C + N + 2
    fmat_d = nc.dram_tensor("fmat", [128, FM], mmdt, kind="ExternalInput").ap()
    # aux = [ bias5 (5) | ohK (40) | Dab (40) | r0row (5) | drow (5) |
    #         R1 (20) | R2 (20) | ohD40 (40) ]  = 175 cols
    AUXW = NCH + NCH * C + NCH * C + NCH + NCH + 4 * NCH + 4 * NCH + NCH * C
    aux_d = nc.dram_tensor("aux", [128, AUXW], f32, kind="ExternalInput").ap()
    out_d = nc.dram_tensor("out", [128, NCH * 2], f32, kind="ExternalOutput").ap()

    # class-pair groups -> one PSUM tile each (pair width ~1030 -> 3 banks)
    groups = [(offs[2 * t], offs[2 * t + 2]) for t in range(4)]

    with ExitStack() as ctx:
        tc = ctx.enter_context(tile.TileContext(nc))
        const = ctx.enter_context(tc.tile_pool(name="const", bufs=1))
        epool = ctx.enter_context(tc.tile_pool(name="epool", bufs=2))
        psumL = ctx.enter_context(tc.tile_pool(name="psumL", bufs=2, space="PSUM"))
        psumT = ctx.enter_context(tc.tile_pool(name="psumT", bufs=2, space="PSUM"))
        small = ctx.enter_context(tc.tile_pool(name="small", bufs=3))

        # warm the exp table set while DMAs stream in
        warm = const.tile([1, 2], f32)
        nc.vector.memset(warm[:], 0.0)
        nc.scalar.activation(warm[:], warm[:], ACT.Exp, bias=0.0, scale=1.0)

        # matmul operands first (gate the pipeline start), then per-row consts
        fmat = const.tile([128, FM], mmdt)
        cut = RPAD + C + offs[2]
        nc.sync.dma_start(fmat[:, 0:cut], fmat_d[:, 0:cut])
        aux = const.tile([128, AUXW], f32)
        nc.sync.dma_start(aux[:], aux_d)
        for t in range(1, 4):
            a_, b_ = RPAD + C + offs[2 * t], RPAD + C + offs[2 * t + 2]
            if t == 3:
                b_ = FM
            nc.sync.dma_start(fmat[:, a_:b_], fmat_d[:, a_:b_])
        lhsT10 = fmat[:, 0:RPAD]
        GT = fmat[:, RPAD:RPAD + C]
        featsT = fmat[:, RPAD + C:FM]   # [128, N+2], pad included in DMA

        o = 0
        bias5 = aux[:, o:o + NCH]; o += NCH
        ohK = aux[:, o:o + NCH * C]; o += NCH * C
        Dab = aux[:, o:o + NCH * C]; o += NCH * C
        r0row = aux[:, o:o + NCH]; o += NCH
        drow = aux[:, o:o + NCH]; o += NCH
        R1 = aux[:, o:o + 4 * NCH]; o += 4 * NCH
        R2 = aux[:, o:o + 4 * NCH]; o += 4 * NCH
        ohD40 = aux[:, o:o + NCH * C]; o += NCH * C

        # eye-replicated [128, 8*128]: eyerep[p, w*128 + j] = (j == p)
        eyei = const.tile([128, 8 * 128], i32)
        nc.gpsimd.iota(eyei[:], pattern=[[0, 8], [1, 128]], base=0,
                       channel_multiplier=-1)
        eyerep = const.tile([128, 8 * 128], f32)
        nc.vector.tensor_scalar(eyerep[:], eyei[:], 0, None, ALU.is_equal)

        out_sb = const.tile([128, NCH * 2], f32)
        Pvall = const.tile([128, NCH * 4], f32)
        SAall = const.tile([128, NCH * 4], f32)
        dcall = const.tile([128, NCH * C], f32)
        Tall = psumT.tile([128, NCH * C], f32, bufs=1)

        for k in range(NCH):
            rowsW = lhsT10[:, k * 128:(k + 1) * 128]

            # strided diag windows read at most 124 columns past N (chunk 4);
            # zero them within this tile so the masked products are exact 0
            ETAIL = 512
            e_sb = epool.tile([128, N + ETAIL], f32, tag="e")
            nc.gpsimd.memset(e_sb[:, N:N + 128], 0.0)

            for t in range(4):
                g0, g1 = groups[t]
                w = g1 - g0
                wpad = w + (w % 2)
                Lt = psumL.tile([128, max(wpad, 2)], f32, tag="L", name=f"L_{k}_{t}")
                s = 0
                while s < w:
                    ww = min(512, w - s)
                    ww += ww % 2  # fp32r needs even moving/dst widths
                    nc.tensor.matmul(
                        Lt[:, s:s + ww],
                        lhsT=rowsW,
                        rhs=featsT[:, g0 + s:g0 + s + ww],
                        start=True,
                        stop=True,
                    )
                    s += ww
                # one exp per pair tile; accum gives the pair sum
                if w > 0:
                    nc.scalar.activation(
                        e_sb[:, g0:g1],
                        Lt[:, 0:w],
                        ACT.Exp,
                        bias=bias5[:, k:k + 1],
                        scale=1.0,
                        accum_out=Pvall[:, k * 4 + t:k * 4 + t + 1],
                    )
                else:
                    nc.vector.memset(Pvall[:, k * 4 + t:k * 4 + t + 1], 0.0)
                # per-pair disentangle: sum of the even class on the vector
                # engine (global column slice, same on every core)
                a, b = offs[2 * t], offs[2 * t + 1]
                if b > a:
                    nc.vector.reduce_sum(
                        SAall[:, k * 4 + t:k * 4 + t + 1], e_sb[:, a:b], axis=AX.X
                    )
                else:
                    nc.vector.memset(SAall[:, k * 4 + t:k * 4 + t + 1], 0.0)

            if stage < 2:
                nc.vector.tensor_copy(
                    out_sb[:, k:k + 1], Pvall[:, k * 4:k * 4 + 1]
                )
                nc.vector.tensor_copy(
                    out_sb[:, NCH + k:NCH + k + 1], SAall[:, k * 4:k * 4 + 1]
                )
                continue

            nc.tensor.matmul(
                Tall[:, k * C:(k + 1) * C], lhsT=rowsW, rhs=GT[:],
                start=True, stop=True,
            )

            # diagonal e values: 8 candidate 128-wide windows at stride RP
            # (window w serves device w; ohD40 selects the own window later)
            dscr = small.tile([128, 1024], f32, tag="dscr")
            if k < NCH - 1:
                e3 = e_sb[:, k * 128:k * 128 + 8 * RP].rearrange(
                    "p (w r) -> p w r", r=RP
                )[:, :, 0:128]
                m3 = eyerep[:].rearrange("p (w r) -> p w r", r=128)
                nc.gpsimd.tensor_tensor(
                    dscr[:].rearrange("p (w r) -> p w r", r=128), e3, m3,
                    op=ALU.mult,
                )
                nc.vector.reduce_sum(
                    dcall[:, k * C:(k + 1) * C],
                    dscr[:].rearrange("p (w r) -> p w r", r=128),
                    axis=AX.X,
                )
            else:
                # last chunk: per-window products+reduces so each overlaps
                # with the still-running exps of later pairs
                for wdx in range(8):
                    nc.vector.tensor_mul(
                        dscr[:, wdx * 128:(wdx + 1) * 128],
                        e_sb[:, k * 128 + RP * wdx:k * 128 + RP * wdx + 128],
                        eyerep[:, wdx * 128:(wdx + 1) * 128],
                    )
                    nc.vector.reduce_sum(
                        dcall[:, k * C + wdx:k * C + wdx + 1],
                        dscr[:, wdx * 128:(wdx + 1) * 128],
                        axis=AX.X,
                    )

        # ---- batched epilogue over all 5 chunks ----
        # u_k = sum_c Tall[k,c] * ohK[k,c]
        Tsb = const.tile([128, NCH * C], f32)
        nc.vector.tensor_copy(Tsb[:], Tall[:])
        scrU = const.tile([128, NCH * C], f32)
        nc.vector.tensor_mul(scrU[:], Tsb[:], ohK[:])
        nc.vector.reduce_sum(
            out_sb[:, 0:NCH],
            scrU[:].rearrange("p (k c) -> p k c", c=C),
            axis=AX.X,
        )
        # per-class sums: even = SAall, odd = Pvall - SAall (exact zeros)
        Poall = const.tile([128, NCH * 4], f32)
        nc.vector.tensor_sub(Poall[:], Pvall[:], SAall[:])
        scrA = const.tile([128, NCH * 4], f32)
        nc.vector.tensor_mul(scrA[:], SAall[:], R1[:])
        scrB = const.tile([128, NCH * 4], f32)
        nc.vector.tensor_mul(scrB[:], Poall[:], R2[:])
        q1a = const.tile([128, NCH], f32)
        nc.vector.reduce_sum(q1a[:], scrA[:].rearrange("p (k c) -> p k c", c=4),
                             axis=AX.X)
        q1b = const.tile([128, NCH], f32)
        nc.vector.reduce_sum(q1b[:], scrB[:].rearrange("p (k c) -> p k c", c=4),
                             axis=AX.X)
        Dab_r = Dab.rearrange("p (k c) -> p k c", c=C)
        scrC = const.tile([128, NCH * 4], f32)
        nc.vector.tensor_tensor(
            scrC[:].rearrange("p (k c) -> p k c", c=4),
            SAall[:].rearrange("p (k c) -> p k c", c=4),
            Dab_r[:, :, 0:4], op=ALU.mult,
        )
        scrD = const.tile([128, NCH * 4], f32)
        nc.vector.tensor_tensor(
            scrD[:].rearrange("p (k c) -> p k c", c=4),
            Poall[:].rearrange("p (k c) -> p k c", c=4),
            Dab_r[:, :, 4:8], op=ALU.mult,
        )
        q2a = const.tile([128, NCH], f32)
        nc.vector.reduce_sum(q2a[:], scrC[:].rearrange("p (k c) -> p k c", c=4),
                             axis=AX.X)
        q2b = const.tile([128, NCH], f32)
        nc.vector.reduce_sum(q2b[:], scrD[:].rearrange("p (k c) -> p k c", c=4),
                             axis=AX.X)
        # diag pick
        dselall = const.tile([128, NCH * C], f32)
        nc.vector.tensor_mul(dselall[:], dcall[:], ohD40[:])
        dvall = const.tile([128, NCH], f32)
        nc.vector.reduce_sum(dvall[:], dselall[:].rearrange("p (k c) -> p k c", c=C),
                             axis=AX.X)
        # denom_k = (q1 - r0row*dv) + (q2 - drow*dv)
        t1 = const.tile([128, NCH], f32)
        nc.vector.tensor_mul(t1[:], dvall[:], r0row[:])
        t2 = const.tile([128, NCH], f32)
        nc.vector.tensor_mul(t2[:], dvall[:], drow[:])
        q1s = const.tile([128, NCH], f32)
        nc.vector.tensor_add(q1s[:], q1a[:], q1b[:])
        q2s = const.tile([128, NCH], f32)
        nc.vector.tensor_add(q2s[:], q2a[:], q2b[:])
        e1 = const.tile([128, NCH], f32)
        nc.vector.tensor_sub(e1[:], q1s[:], t1[:])
        e2 = const.tile([128, NCH], f32)
        nc.vector.tensor_sub(e2[:], q2s[:], t2[:])
        nc.vector.tensor_tensor(out_sb[:, NCH:NCH * 2], e1[:], e2[:], op=ALU.add)

        nc.sync.dma_start(out_d, out_sb[:])

    nc.compile()
    return nc


def _lay(arr):
    """[640, M] per-slot array -> [128, 5*M] chunk-major SBUF layout."""
    m = arr.shape[1]
    return np.ascontiguousarray(
        arr.reshape(NCH, 128, m).transpose(1, 0, 2).reshape(128, NCH * m)
    )


def _prepare(proxy: np.ndarray, features: np.ndarray, targets: np.ndarray):
    proxy = np.asarray(proxy, dtype=np.float32)
    features = np.asarray(features, dtype=np.float32)
    targets_i = np.asarray(targets).astype(np.int64)

    proxy_t = np.repeat(np.arange(C), PROXY_NUM_LIST)
    t_all = np.concatenate([targets_i, targets_i, proxy_t]).astype(np.int64)
    feats = np.concatenate([features[:, 0], features[:, 1], proxy], axis=0).astype(
        np.float32
    )
    assert feats.shape == (N, D)

    perm = np.argsort(t_all, kind="stable")
    t_s = t_all[perm]
    feats_s = np.ascontiguousarray(feats[perm])
    cnt = np.bincount(t_s, minlength=C).astype(np.int64)
    offs = np.zeros(C + 1, dtype=np.int64)
    offs[1:] = np.cumsum(cnt)
    offs_t = tuple(int(x) for x in offs)

    featsT = np.ascontiguousarray(feats_s.T)                       # [128, N]
    lhsT10_full = np.ascontiguousarray((feats_s * np.float32(10.0)).T)
    G = np.stack(
        [feats_s[offs[c]:offs[c + 1]].sum(axis=0, dtype=np.float32) for c in range(C)]
    )
    GT = np.ascontiguousarray(G.T.astype(np.float32))              # [128, C]

    self10 = (feats_s * feats_s).sum(axis=1, dtype=np.float32) * np.float32(10.0)
    cntf = cnt.astype(np.float32)
    with np.errstate(divide="ignore", invalid="ignore"):
        r0 = np.float32(1.0) / cntf                                # 1/cnt
        r1 = np.float32(1.0) / (cntf - np.float32(1.0))            # 1/(cnt-1)
        dr = (r1 - r0).astype(np.float32)
        kap = r1

    # global per-pair constants, tiled per chunk: [128, 20]
    R1v = np.tile(r0[0::2], NCH).astype(np.float32)
    R2v = np.tile(r0[1::2], NCH).astype(np.float32)
    R1_bc = np.ascontiguousarray(np.broadcast_to(R1v, (128, 4 * NCH)))
    R2_bc = np.ascontiguousarray(np.broadcast_to(R2v, (128, 4 * NCH)))

    # per-core input maps
    in_maps = []
    host_meta = []
    for d in range(NCORES):
        gidx = np.concatenate(
            [np.arange(RP * d, RP * (d + 1)), np.full(RPAD - RP, RP * d)]
        ).astype(np.int64)
        valid = np.arange(RPAD) < RP
        cls = t_s[gidx]

        lhsT10_dev = np.ascontiguousarray(lhsT10_full[:, gidx])

        bias5 = (-self10[gidx][:, None]).astype(np.float32)        # [640, 1]
        oh = (np.arange(C)[None, :] == cls[:, None]).astype(np.float32)
        ohK = (kap[cls][:, None] * oh).astype(np.float32)
        ohK[~valid] = 0.0
        # Dab[:, 0:4] -> weight on SA_t (even class of pair t)
        # Dab[:, 4:8] -> weight on Po_t (odd class of pair t)
        Dab = np.zeros((RPAD, C), dtype=np.float32)
        drc = dr[cls]
        for t in range(4):
            Dab[:, t] = np.where(cls == 2 * t, drc, 0.0)
            Dab[:, 4 + t] = np.where(cls == 2 * t + 1, drc, 0.0)
        Dab[~valid] = 0.0
        r0row = np.where(valid, r0[cls], np.float32(0)).astype(np.float32)[:, None]
        drow = np.where(valid, dr[cls], np.float32(0)).astype(np.float32)[:, None]

        ohDv = np.tile((np.arange(C) == d).astype(np.float32), NCH)
        ohD_bc = np.ascontiguousarray(np.broadcast_to(ohDv, (128, NCH * C)))

        # fmat = [ lhsT10 | GT | featsT | pad(2 zero cols) ]
        fmat = np.concatenate(
            [lhsT10_dev, GT, featsT, np.zeros((128, 2), np.float32)], axis=1
        )
        aux = np.concatenate(
            [
                _lay(bias5),
                _lay(ohK),
                _lay(Dab),
                _lay(r0row),
                _lay(drow),
                R1_bc,
                R2_bc,
                ohD_bc,
            ],
            axis=1,
        )
        in_maps.append(
            {
                "fmat": np.ascontiguousarray(fmat),
                "aux": np.ascontiguousarray(aux),
            }
        )
        host_meta.append((gidx, valid, cls))

    import os

    fp32r = os.environ.get("KERNEL_FP32R", "1") == "1"
    stage = int(os.environ.get("KERNEL_STAGE", "4"))
    key = (offs_t, fp32r, stage)
    nc = _prog_cache.get(key)
    if nc is None:
        nc = _build_program(offs_t, fp32r=fp32r, stage=stage)
        _prog_cache[key] = nc

    return nc, in_maps, host_meta, t_s, cnt, kap, self10


def _assemble(results, host_meta, t_s, cnt, kap, self10):
    # host epilogue: mean_log_prob_pos per sorted row, reference fp32 semantics
    mlpp = np.zeros(N, dtype=np.float64)
    with np.errstate(divide="ignore", invalid="ignore"):
        for d in range(NCORES):
            O = results[d]["out"]                     # [128, 10]
            gidx, valid, cls = host_meta[d]
            for k in range(NCH):
                u = O[:, k].astype(np.float32)
                den = O[:, NCH + k].astype(np.float32)
                srows = np.arange(k * 128, (k + 1) * 128)
                vs = srows[valid[srows]]
                g = gidx[vs]
                uk = u[vs - k * 128]
                dk = den[vs - k * 128]
                a = kap[t_s[g]]
                logd = np.log(dk)
                val = (
                    uk
                    - a * cnt[t_s[g]].astype(np.float32) * self10[g]
                    - logd
                    + np.float32(0.0) * logd
                )
                mlpp[g] = val
    loss = -np.float32(np.mean(mlpp))
    return np.float32(loss)


def kernel(proxy: np.ndarray, features: np.ndarray, targets: np.ndarray) -> np.ndarray:
    import concourse.bass_utils as bass_utils

    nc, in_maps, host_meta, t_s, cnt, kap, self10 = _prepare(proxy, features, targets)
    res = bass_utils.run_bass_kernel_spmd(nc, in_maps, core_ids=list(range(NCORES)))
    return _assemble(res.results, host_meta, t_s, cnt, kap, self10)
